# revision 23
# baseline (speedup 1.0000x reference)
"""Trainium2 Bass kernel for nn_CaptionEmbedding (ragged double-GRU with
attention gating).

Strategy: data-parallel over batch across 8 cores (strided over the
length-sorted order so every core gets a balanced length mix). Per core a
fully-unrolled 20-step recurrence in fp16 (fp32 PSUM accumulation):
  - activations "stacked": [128, 512] = (slot + 64*feat_half, feat%512)
  - matmul stationary operands are activations (64-wide -> column-tiled
    T0/T1 pairs run concurrently); weights stream through the PE array
  - ALL weights resident in SBUF fp16 (whh, wcih, wcwh, wh, wl, wf)
  - w-GRU input projections (gi = x @ Wih^T) for all 20 steps are
    precomputed in the prologue with a 128-wide stationary (full PE
    utilization), bounced through a DRAM scratch, and streamed back one
    step per slot (3 KiB/partition)
  - the c-GRU chain (cgh/cgi/Wf) runs one step behind the w-GRU/attention
    chain so the PE has independent work during the recurrent combines
"""
import numpy as np

import concourse.bass as bass
import concourse.mybir as mybir
import concourse.tile as tile
from concourse.bass_utils import run_bass_kernel_spmd
import concourse.mybir as _mybir

B, VD, QD, HD, L = 512, 2048, 1024, 1024, 20
NCORES, S = 8, 64
NPAIR = L // 2
F32, F16 = mybir.dt.float32, mybir.dt.float16
Sig = mybir.ActivationFunctionType.Sigmoid
Tanh = mybir.ActivationFunctionType.Tanh
Relu = mybir.ActivationFunctionType.Relu
Copy = mybir.ActivationFunctionType.Copy

_MAX_WAITS = 1
_wait_ctr = [0]


def _dedupe_ldw(nc):
    """Tile legalization emits one InstLdweights per matmul; consecutive
    matmuls over the same stationary reload identical weights. Drop exact
    duplicates (no sync side effects) so the PE streams back-to-back."""
    import concourse.mybir as mb

    dropped = 0
    for fn in nc.m.functions:
        for bb in fn.blocks:
            out = []
            last = {}  # (tile_position, tile_size) -> weights key
            for inst in bb.instructions:
                nm = type(inst).__name__
                if nm == "InstLdweights":
                    si = inst.sync_info
                    pos = (
                        tuple(getattr(inst, "tile_position", None) or (-1,)),
                        tuple(getattr(inst, "tile_size", None) or (-1,)),
                    )
                    key = (
                        str(inst.ins[0]),
                        bool(getattr(inst, "is_transpose", False)),
                        str(getattr(inst, "perf_mode", None)),
                    )
                    clean = not (si and (si.on_wait or si.on_update))
                    if clean and last.get(pos) == key:
                        dropped += 1
                        continue
                    last[pos] = key
                elif nm == "InstMatmult":
                    pass
                elif inst.engine == mb.EngineType.PE:
                    last.clear()
                out.append(inst)
            if len(out) != len(bb.instructions):
                bb.instructions[:] = out
    return dropped


def _split_waits(nc):
    # container neuronxcc rejects >= 2 sync waits on one instruction; move
    # extras onto same-engine nops spliced just before it
    for fn in nc.m.functions:
        for bb in fn.blocks:
            out = []
            for inst in bb.instructions:
                si = inst.sync_info
                waits = list(si.on_wait) if si and si.on_wait else []
                if len(waits) > _MAX_WAITS:
                    extra, keep = waits[:-_MAX_WAITS], waits[-_MAX_WAITS:]
                    for i in range(0, len(extra), _MAX_WAITS):
                        _wait_ctr[0] += 1
                        nop = _mybir.InstNoOp(
                            name=f"waitsplit_nop_{_wait_ctr[0]}", ins=[], outs=[]
                        )
                        nop.engine = inst.engine
                        nop.sync_info = _mybir.SyncInfo(
                            on_wait=extra[i : i + _MAX_WAITS], on_update=[]
                        )
                        nc.register_instruction(nop)
                        out.append(nop)
                    si.on_wait = keep
                out.append(inst)
            if len(out) != len(bb.instructions):
                bb.instructions[:] = out


def _kt_slice(tT, kt):
    # stationary [128, 64] for feature ktile kt from a transposed
    # [128, 4, 128] tile: tT[p, j, q] = stacked[q, j*128 + p]
    hi, j = kt // 4, kt % 4
    return tT[:, j, 64 * hi : 64 * hi + 64]


def _build():
    """Trace the per-core program (identical for all cores; SPMD)."""
    nc = bass.Bass("TRN2", dynamic_dma_scratch_size=64)
    di = {}
    inputs = [
        ("vT", [128, 16, S], F16),
        ("qT", [128, 8, S], F16),
        ("xT2", [NPAIR, 128, 8, 128], F16),
        ("wvT", [16, 128, HD], F16),
        ("wqT", [8, 128, HD], F16),
        ("wihT", [128, 8, 3 * HD], F16),
        ("whhT", [128, 8, 3 * HD], F16),
        ("whT", [128, 8, HD], F16),
        ("wlT", [128, 8, HD], F16),
        ("wcihT", [128, 8, 3 * HD], F16),
        ("wcwhT", [128, 8, 3 * HD], F16),
        ("wfT", [128, 8, HD], F16),
    ]
    for name, shape, dt in inputs:
        di[name] = nc.dram_tensor(name, shape, dt, kind="ExternalInput")
    outs_d = nc.dram_tensor("outs", [L, 128, 512], F32, kind="ExternalOutput")
    alph_d = nc.dram_tensor("alph", [L, 128, 512], F16, kind="ExternalOutput")

    with tile.TileContext(nc) as tc:
        _trace(nc, tc, di, outs_d, alph_d)
    _split_waits(nc)
    return nc


def _trace(nc, tc, di, outs_d, alph_d):
    import contextlib

    ctx = contextlib.ExitStack()
    with ctx:
        # ---- resident weights, phase 1 ----
        res1 = ctx.enter_context(tc.tile_pool(name="res1", bufs=1))
        whh_t = res1.tile([128, 8, 3 * HD], F16, tag="whh")
        wh_t = res1.tile([128, 8, HD], F16, tag="wh")
        wl_t = res1.tile([128, 8, HD], F16, tag="wl")

        def gp_load(w_sb, dname):
            # bulk weights ride the otherwise-idle gpsimd ring, kt-major
            # in 32-partition sub-chunks (Pool DMA descriptor limit) so
            # consumers of early ktiles start as soon as chunks land
            for kt in range(8):
                for p4 in range(4):
                    sl = slice(32 * p4, 32 * p4 + 32)
                    nc.gpsimd.dma_start(out=w_sb[sl, kt, :],
                                        in_=di[dname][sl, kt, :])

        # ---- DRAM scratch for precomputed gi ----
        dram = ctx.enter_context(tc.tile_pool(name="dram", bufs=1, space="DRAM"))
        gi_dram = dram.tile([L, 128, 3, 512], F16, tag="gi_dram")

        # ---- work pool ----
        work = ctx.enter_context(tc.tile_pool(name="work", bufs=1))
        ctr = [0]

        def wtile(shape, dt, tag, bufs):
            def mk():
                ctr[0] += 1
                return work.tile(shape, dt, tag=tag, bufs=bufs,
                                 name=f"{tag}_{ctr[0]}")
            return mk

        t_xt2 = wtile([128, 8, 128], F16, "xt2", 2)
        t_gi = wtile([128, 3, 512], F16, "gi", 2)
        t_tmp = wtile([128, 512], F16, "tmp", 4)
        t_act = wtile([128, 512], F16, "act", 5)
        t_rw = t_zw = t_nw = t_rc = t_zc = t_nct = t_act
        t_jrl = t_grc = t_att = t_act
        t_h1 = wtile([128, 512], F16, "h1", 1)
        t_h2 = wtile([128, 512], F16, "h2", 1)
        t_h1T = wtile([128, 4, 128], F16, "h1T", 1)
        t_h2T = wtile([128, 4, 128], F16, "h2T", 1)
        t_jT = wtile([128, 4, 128], F16, "jT", 1)
        t_attT = wtile([128, 4, 128], F16, "attT", 1)
        t_grcT = wtile([128, 4, 128], F16, "grcT", 1)
        t_cinT = wtile([128, 4, 128], F16, "cinT", 1)
        t_h2n = wtile([128, 512], F32, "h2n", 1)

        pvq_t = work.tile([128, 512], F16, tag="pvq")
        ident = work.tile([128, 128], F16, tag="ident")
        from concourse.masks import make_identity
        make_identity(nc, ident)

        # ---- prologue: pvq + batched gi -> DRAM ----
        with tc.tile_pool(name="pre", bufs=1) as pre:
            wih_t = pre.tile([128, 8, 3 * HD], F16, tag="wih")
            gp_load(wih_t, "wihT")
            gp_load(whh_t, "whhT")
            gp_load(wh_t, "whT")
            gp_load(wl_t, "wlT")
            v_t = pre.tile([128, 16, S], F16, tag="v")
            q_t = pre.tile([128, 8, S], F16, tag="q")
            nc.scalar.dma_start(out=v_t, in_=di["vT"][:])
            nc.scalar.dma_start(out=q_t, in_=di["qT"][:])

            def t_stage():
                ctr[0] += 1
                return pre.tile([128, 512], F16, tag="stage", bufs=4,
                                name=f"stage_{ctr[0]}")

            with tc.tile_pool(name="preps", bufs=1, space="PSUM") as preps:
                pv = preps.tile([128, 512], F32, tag="pv")
                for kt in range(16):
                    wc = pre.tile([128, HD], F16, tag="wvq", bufs=2,
                                  name=f"wv_{kt}")
                    nc.scalar.dma_start(out=wc, in_=di["wvT"][kt])
                    nc.tensor.matmul(pv[0:64, :], v_t[:, kt, :], wc[:, 0:512],
                                     start=(kt == 0), stop=False)
                    nc.tensor.matmul(pv[64:128, :], v_t[:, kt, :],
                                     wc[:, 512:1024], start=(kt == 0),
                                     stop=False)
                for kt in range(8):
                    wc = pre.tile([128, HD], F16, tag="wvq", bufs=2,
                                  name=f"wq_{kt}")
                    nc.scalar.dma_start(out=wc, in_=di["wqT"][kt])
                    nc.tensor.matmul(pv[0:64, :], q_t[:, kt, :], wc[:, 0:512],
                                     start=False, stop=(kt == 7))
                    nc.tensor.matmul(pv[64:128, :], q_t[:, kt, :],
                                     wc[:, 512:1024], start=False,
                                     stop=(kt == 7))
                nc.vector.tensor_copy(out=pvq_t, in_=pv)

                # batched gi: per pair of steps, 128-wide stationary
                xp0 = t_xt2()
                nc.sync.dma_start(out=xp0, in_=di["xT2"][0])
                xp_prev = xp0
                for p in range(NPAIR):
                    xp = xp_prev
                    if p + 1 < NPAIR:
                        xp_prev = t_xt2()
                        nc.sync.dma_start(out=xp_prev, in_=di["xT2"][p + 1])
                    ps6 = []
                    for i in range(6):
                        ctr[0] += 1
                        ps6.append(preps.tile([128, 512], F32, tag="pg",
                                              bufs=7, name=f"pg_{ctr[0]}"))
                    for kt in range(8):
                        lhsT = xp[:, kt, :]
                        st, sp = kt == 0, kt == 7
                        for i in range(6):
                            nc.tensor.matmul(
                                ps6[i], lhsT,
                                wih_t[:, kt, 512 * i : 512 * (i + 1)],
                                start=st, stop=sp)
                    for g in range(3):
                        lo, hi = ps6[2 * g], ps6[2 * g + 1]
                        slo = t_stage()
                        nc.vector.tensor_copy(out=slo, in_=lo)
                        shi = t_stage()
                        nc.vector.tensor_copy(out=shi, in_=hi)
                        for s2 in range(2):
                            t = 2 * p + s2
                            nc.sync.dma_start(
                                out=gi_dram[t, 0:64, g, :],
                                in_=slo[64 * s2 : 64 * s2 + 64, :])
                            nc.sync.dma_start(
                                out=gi_dram[t, 64:128, g, :],
                                in_=shi[64 * s2 : 64 * s2 + 64, :])

        # ---- resident weights, phase 2 (after prologue pool freed) ----
        res2 = ctx.enter_context(tc.tile_pool(name="res2", bufs=1))
        wcwh_t = res2.tile([128, 8, 3 * HD], F16, tag="wcwh")
        wcih_t = res2.tile([128, 8, 3 * HD], F16, tag="wcih")
        wf_t = res2.tile([128, 8, HD], F16, tag="wf")

        # ---- main-loop PSUM pool ----
        psum = ctx.enter_context(tc.tile_pool(name="psum", bufs=1,
                                              space="PSUM"))

        def gate_ps():
            ctr[0] += 1
            return psum.tile([128, 512], F32, tag="pg", bufs=5,
                             name=f"ps_{ctr[0]}")

        def rot_ps():
            ctr[0] += 1
            return psum.tile([128, 512], F32, tag="rot", bufs=2,
                             name=f"rot_{ctr[0]}")

        def pt_ps():
            ctr[0] += 1
            return psum.tile([128, 512], F16, tag="psT", bufs=1,
                             name=f"psT_{ctr[0]}")

        def pe_transpose(dstT, src_f16, evac):
            # dstT [128, 4, 128] <- transpose of stacked [128, 512] fp16
            pt = pt_ps()
            for j in range(4):
                nc.tensor.transpose(
                    pt[:, 128 * j : 128 * (j + 1)],
                    src_f16[:, 128 * j : 128 * (j + 1)],
                    ident,
                )
            if evac == "vector":
                nc.vector.tensor_copy(
                    out=dstT.rearrange("p j q -> p (j q)"), in_=pt)
            else:
                nc.scalar.activation(
                    out=dstT.rearrange("p j q -> p (j q)"), in_=pt, func=Copy)

        def mm_gates(psums, statT, w_t, start, stops):
            """Per ktile: pairs of col-tiled matmuls for each 512-wide
            output slice i of psums (accumulating over ktiles). `stops`
            is a per-psum list of whether the group closes at kt 7."""
            if stops in (True, False):
                stops = [stops] * len(psums)
            for kt in range(8):
                lhsT = _kt_slice(statT, kt)
                st = start and kt == 0
                for i, ps in enumerate(psums):
                    sp = stops[i] and kt == 7
                    nc.tensor.matmul(
                        ps[0:64, :], lhsT,
                        w_t[:, kt, 1024 * i : 1024 * i + 512],
                        start=st, stop=sp)
                    nc.tensor.matmul(
                        ps[64:128, :], lhsT,
                        w_t[:, kt, 1024 * i + 512 : 1024 * i + 1024],
                        start=st, stop=sp)

        # ---- initial state ----
        h1 = t_h1()
        nc.vector.memset(h1, 0.0)
        h1T_prev = t_h1T()
        nc.vector.memset(h1T_prev, 0.0)
        h2 = t_h2()
        nc.vector.memset(h2, 0.0)
        h2T_prev = t_h2T()
        nc.vector.memset(h2T_prev, 0.0)

        gi_tiles, xp_tiles = {}, {}

        def load_gi(t):
            g = t_gi()
            nc.sync.dma_start(out=g, in_=gi_dram[t])
            gi_tiles[t] = g

        def load_xp(p):
            xp = t_xt2()
            nc.sync.dma_start(out=xp, in_=di["xT2"][p])
            xp_tiles[p] = xp

        # gi(0)/xp(0) queued ahead of the phase-2 weight loads so the
        # first slot's inputs land first. Phase-2 weights stream in
        # per-ktile chunks across two rings so consumers (E/D/F of the
        # first c-slot) start as soon as their chunk lands.
        load_gi(0)
        load_xp(0)
        gp_load(wcwh_t, "wcwhT")
        gp_load(wcih_t, "wcihT")
        gp_load(wf_t, "wfT")

        cin_prev = None  # cinT for step k-1
        c_ps = None      # (Rc, Zc, HNc) psums for c-step in flight

        # ---- main loop: slot k handles h1-chain step k, c-chain k-1 ----
        for k in range(L + 1):
            hk, ck = k, k - 1
            do_h, do_c = hk < L, ck >= 0

            # prefetches for next slot
            if hk + 1 < L:
                load_gi(hk + 1)
                if (hk + 1) % 2 == 0:
                    load_xp((hk + 1) // 2)

            # === A(k): w-GRU hidden projections ===
            if do_h:
                Rw, Zw, HNw = gate_ps(), gate_ps(), gate_ps()
                mm_gates([Rw, Zw, HNw], h1T_prev, whh_t, True, True)

            # === E(ck): c-GRU hidden projections (group left open) ===
            if do_c:
                Rc, Zc, HNc = gate_ps(), gate_ps(), gate_ps()
                mm_gates([Rc, Zc, HNc], h2T_prev, wcwh_t, True,
                         [False, False, True])

            # === V1(k): w-GRU combine ===
            if do_h:
                git = gi_tiles.pop(hk)
                rs = t_tmp()
                nc.vector.tensor_add(out=rs, in0=Rw, in1=git[:, 0, :])
                zs = t_tmp()
                nc.vector.tensor_add(out=zs, in0=Zw, in1=git[:, 1, :])
                rw = t_rw()
                nc.scalar.activation(out=rw, in_=rs, func=Sig)
                zw = t_zw()
                nc.scalar.activation(out=zw, in_=zs, func=Sig)
                t1 = t_tmp()
                nc.vector.tensor_mul(out=t1, in0=rw, in1=HNw)
                t2 = t_tmp()
                nc.vector.tensor_add(out=t2, in0=t1, in1=git[:, 2, :])
                nw = t_nw()
                nc.scalar.activation(out=nw, in_=t2, func=Tanh)
                d = t_tmp()
                nc.vector.tensor_sub(out=d, in0=h1, in1=nw)
                e = t_tmp()
                nc.vector.tensor_mul(out=e, in0=zw, in1=d)
                h1n = t_h1()
                nc.vector.tensor_add(out=h1n, in0=nw, in1=e)
                h1 = h1n

                # === T1(k) + B(k): joint = relu(pvq + h1 @ Wh.T) ===
                h1T_new = t_h1T()
                pe_transpose(h1T_new, h1, evac="vector")
                joint = rot_ps()
                mm_gates([joint], h1T_new, wh_t, True, True)

            # === D(ck): c-GRU input projections (close Rc/Zc groups) ===
            if do_c:
                INc = gate_ps()
                for kt in range(8):
                    lhsT = _kt_slice(cin_prev, kt)
                    sp = kt == 7
                    nc.tensor.matmul(Rc[0:64, :], lhsT,
                                     wcih_t[:, kt, 0:512],
                                     start=False, stop=sp)
                    nc.tensor.matmul(Rc[64:128, :], lhsT,
                                     wcih_t[:, kt, 512:1024],
                                     start=False, stop=sp)
                    nc.tensor.matmul(Zc[0:64, :], lhsT,
                                     wcih_t[:, kt, 1024:1536],
                                     start=False, stop=sp)
                    nc.tensor.matmul(Zc[64:128, :], lhsT,
                                     wcih_t[:, kt, 1536:2048],
                                     start=False, stop=sp)
                    nc.tensor.matmul(INc[0:64, :], lhsT,
                                     wcih_t[:, kt, 2048:2560],
                                     start=(kt == 0), stop=sp)
                    nc.tensor.matmul(INc[64:128, :], lhsT,
                                     wcih_t[:, kt, 2560:3072],
                                     start=(kt == 0), stop=sp)

            if do_h:
                # === V2(k) + T2(k) + C(k): att = sig(relu(...) @ Wl.T) ===
                ja = t_tmp()
                nc.vector.tensor_add(out=ja, in0=joint, in1=pvq_t)
                jrl = t_jrl()
                nc.scalar.activation(out=jrl, in_=ja, func=Relu)
                jT = t_jT()
                pe_transpose(jT, jrl, evac="vector")
                attp = rot_ps()
                mm_gates([attp], jT, wl_t, True, True)

            # === V5(ck): c-GRU combine ===
            if do_c:
                rc = t_rc()
                nc.scalar.activation(out=rc, in_=Rc, func=Sig)
                zc = t_zc()
                nc.scalar.activation(out=zc, in_=Zc, func=Sig)
                t1c = t_tmp()
                nc.vector.tensor_mul(out=t1c, in0=rc, in1=HNc)
                t2c = t_tmp()
                nc.vector.tensor_add(out=t2c, in0=t1c, in1=INc)
                nct = t_nct()
                nc.scalar.activation(out=nct, in_=t2c, func=Tanh)
                dc = t_tmp()
                nc.vector.tensor_sub(out=dc, in0=h2, in1=nct)
                ec = t_tmp()
                nc.vector.tensor_mul(out=ec, in0=zc, in1=dc)
                grc = t_grc()
                nc.vector.tensor_add(out=grc, in0=nct, in1=ec)

            if do_h:
                # === V3(k) + T3(k) + V4(k): att, cin ===
                att = t_att()
                nc.scalar.activation(out=att, in_=attp, func=Sig)
                nc.sync.dma_start(out=alph_d[hk], in_=att)
                attT = t_attT()
                pe_transpose(attT, att, evac="scalar")
                cinT = t_cinT()
                xp = xp_tiles[hk // 2]
                s2 = hk % 2
                xs = xp[:, :, 64 * s2 : 64 * s2 + 64]
                xt_r = xs.rearrange("p (hi j) s -> p j hi s", hi=2, j=4)
                nc.vector.tensor_mul(
                    out=cinT.rearrange("p j (hi s) -> p j hi s", hi=2),
                    in0=attT.rearrange("p j (hi s) -> p j hi s", hi=2),
                    in1=xt_r,
                )
                cin_new = cinT

            # === T4(ck) + F(ck): h2n = grc @ Wf.T ===
            if do_c:
                grcT = t_grcT()
                pe_transpose(grcT, grc, evac="vector")
                h2np = rot_ps()
                mm_gates([h2np], grcT, wf_t, True, True)
                h2n_sb = t_h2n()
                nc.vector.tensor_copy(out=h2n_sb, in_=h2np)
                nc.sync.dma_start(out=outs_d[ck], in_=h2n_sb)
                if ck < L - 1:
                    h2n = t_h2()
                    nc.scalar.activation(out=h2n, in_=h2np, func=Copy)
                    h2 = h2n
                    h2Tn = t_h2T()
                    pe_transpose(h2Tn, h2, evac="scalar")
                    h2T_prev = h2Tn

            if do_h:
                h1T_prev = h1T_new
                cin_prev = cin_new


_CACHED = {}


def _get_nc():
    if "nc" not in _CACHED:
        _CACHED["nc"] = _build()
    return _CACHED["nc"]


def _wn(V, g):
    return V * (g / np.linalg.norm(V.astype(np.float64)).astype(np.float32))


def _plainT(W):
    # [out, in] -> [in//128, 128, out] fp16
    inf = W.shape[1]
    return np.ascontiguousarray(W.T.reshape(inf // 128, 128, W.shape[0])).astype(
        np.float16
    )


def _plain128(W):
    # [out, in] -> [128, in//128, out] fp16 (partition-major, 1 DMA)
    return np.ascontiguousarray(np.transpose(_plainT(W), (1, 0, 2)))


def _prep_in_maps(inp):
    cap_len = inp["cap_len"].astype(np.int32)
    order = np.argsort(-cap_len, kind="stable")

    for bname in ["av_b", "aq_b", "ah_b", "al_b", "fc_b",
                  "w_bih", "w_bhh", "c_bih", "c_bhh"]:
        assert not np.any(inp[bname]), f"nonzero bias {bname} unsupported"

    Wv = _wn(inp["av_V"], inp["av_g"])
    Wq = _wn(inp["aq_V"], inp["aq_g"])
    Wh = _wn(inp["ah_V"], inp["ah_g"])
    Wl = _wn(inp["al_V"], inp["al_g"])
    Wf = _wn(inp["fc_V"], inp["fc_g"])

    shared = dict(
        wvT=_plainT(Wv), wqT=_plainT(Wq),
        wihT=_plain128(inp["w_Wih"]),
        whhT=_plain128(inp["w_Whh"]),
        whT=_plain128(Wh), wlT=_plain128(Wl),
        wcihT=_plain128(inp["c_Wih"]), wcwhT=_plain128(inp["c_Whh"]),
        wfT=_plain128(Wf),
    )

    v, q, caption = inp["v"], inp["q"], inp["caption"]
    in_maps = []
    for k in range(NCORES):
        pos = np.arange(S) * NCORES + k  # sorted positions of this core
        vk = v[pos].astype(np.float16)            # [S, VD]
        qk = q[pos].astype(np.float16)
        capk = caption[order[pos]].astype(np.float16)  # [S, L, QD]
        m = dict(shared)
        m["vT"] = np.ascontiguousarray(
            np.transpose(vk.T.reshape(16, 128, S), (1, 0, 2)))
        m["qT"] = np.ascontiguousarray(
            np.transpose(qk.T.reshape(8, 128, S), (1, 0, 2)))
        # xT2[p, pf, kt, 64*s + b] = cap[b, 2p+s, 128*kt + pf]
        c2 = capk.reshape(S, NPAIR, 2, 8, 128)
        m["xT2"] = np.ascontiguousarray(
            np.transpose(c2, (1, 4, 3, 2, 0)).reshape(NPAIR, 128, 8, 128))
        in_maps.append(m)
    return in_maps


def kernel(**inputs):
    inp = {k: np.asarray(v) for k, v in inputs.items()}
    cap_len = inp["cap_len"].astype(np.int32)
    order = np.argsort(-cap_len, kind="stable")
    cl = cap_len[order]
    in_maps = _prep_in_maps(inp)

    nc = _get_nc()
    res = run_bass_kernel_spmd(nc, in_maps, core_ids=list(range(NCORES)))

    outs = np.zeros((B, L, HD), np.float32)
    alphas = np.zeros((B, L, HD), np.float32)
    for k in range(NCORES):
        pos = np.arange(S) * NCORES + k
        od = res.results[k]["outs"]  # [L, 128, 512] f32
        ad = res.results[k]["alph"].astype(np.float32)
        oc = np.concatenate([od[:, :S, :], od[:, S:, :]], axis=2)  # [L, S, HD]
        ac = np.concatenate([ad[:, :S, :], ad[:, S:, :]], axis=2)
        outs[pos] = np.transpose(oc, (1, 0, 2))
        alphas[pos] = np.transpose(ac, (1, 0, 2))

    mask = (np.arange(L)[None, :] < cl[:, None])[:, :, None]
    outs *= mask
    alphas *= mask
    return outs, alphas


# revision 40
# speedup vs baseline: 1.6871x; 1.6871x over previous
"""Trainium2 Bass kernel for nn_CaptionEmbedding (ragged double-GRU with
attention gating).

Strategy: data-parallel over batch across 8 cores (strided over the
length-sorted order so every core gets a balanced length mix). Per core a
fully-unrolled 20-step recurrence in fp16 (fp32 PSUM accumulation):
  - activations "stacked": [128, 512] = (slot + 64*feat_half, feat%512)
  - matmul stationary operands are activations (64-wide -> column-tiled
    T0/T1 pairs run concurrently); weights stream through the PE array
  - ALL weights resident in SBUF fp16 (whh, wcih, wcwh, wh, wl, wf)
  - w-GRU input projections (gi = x @ Wih^T) for all 20 steps are
    precomputed in the prologue with a 128-wide stationary (full PE
    utilization), bounced through a DRAM scratch, and streamed back one
    step per slot (3 KiB/partition)
  - the c-GRU chain (cgh/cgi/Wf) runs one step behind the w-GRU/attention
    chain so the PE has independent work during the recurrent combines
"""
import numpy as np

import concourse.bass as bass
import concourse.mybir as mybir
import concourse.tile as tile
from concourse.bass_utils import run_bass_kernel_spmd
import concourse.mybir as _mybir

B, VD, QD, HD, L = 512, 2048, 1024, 1024, 20
NCORES, S = 8, 64
NPAIR = L // 2
F32, F16 = mybir.dt.float32, mybir.dt.float16
Sig = mybir.ActivationFunctionType.Sigmoid
Tanh = mybir.ActivationFunctionType.Tanh
Relu = mybir.ActivationFunctionType.Relu
Copy = mybir.ActivationFunctionType.Copy

_MAX_WAITS = 1
_wait_ctr = [0]


def _dedupe_ldw(nc):
    """Tile legalization emits one InstLdweights per matmul; consecutive
    matmuls over the same stationary reload identical weights. Drop exact
    duplicates (no sync side effects) so the PE streams back-to-back."""
    import concourse.mybir as mb

    dropped = 0
    for fn in nc.m.functions:
        for bb in fn.blocks:
            out = []
            last = {}  # (tile_position, tile_size) -> weights key
            for inst in bb.instructions:
                nm = type(inst).__name__
                if nm == "InstLdweights":
                    si = inst.sync_info
                    pos = (
                        tuple(getattr(inst, "tile_position", None) or (-1,)),
                        tuple(getattr(inst, "tile_size", None) or (-1,)),
                    )
                    key = (
                        str(inst.ins[0]),
                        bool(getattr(inst, "is_transpose", False)),
                        str(getattr(inst, "perf_mode", None)),
                    )
                    clean = not (si and (si.on_wait or si.on_update))
                    if clean and last.get(pos) == key:
                        dropped += 1
                        continue
                    last[pos] = key
                elif nm == "InstMatmult":
                    pass
                elif inst.engine == mb.EngineType.PE:
                    last.clear()
                out.append(inst)
            if len(out) != len(bb.instructions):
                bb.instructions[:] = out
    return dropped


def _split_waits(nc):
    # container neuronxcc rejects >= 2 sync waits on one instruction; move
    # extras onto same-engine nops spliced just before it
    for fn in nc.m.functions:
        for bb in fn.blocks:
            out = []
            for inst in bb.instructions:
                si = inst.sync_info
                waits = list(si.on_wait) if si and si.on_wait else []
                if len(waits) > _MAX_WAITS:
                    extra, keep = waits[:-_MAX_WAITS], waits[-_MAX_WAITS:]
                    for i in range(0, len(extra), _MAX_WAITS):
                        _wait_ctr[0] += 1
                        nop = _mybir.InstNoOp(
                            name=f"waitsplit_nop_{_wait_ctr[0]}", ins=[], outs=[]
                        )
                        nop.engine = inst.engine
                        nop.sync_info = _mybir.SyncInfo(
                            on_wait=extra[i : i + _MAX_WAITS], on_update=[]
                        )
                        nc.register_instruction(nop)
                        out.append(nop)
                    si.on_wait = keep
                out.append(inst)
            if len(out) != len(bb.instructions):
                bb.instructions[:] = out


def _kt_slice(tT, kt):
    # stationary [128, 64] for feature ktile kt from a transposed
    # [128, 4, 128] tile: tT[p, j, q] = stacked[q, j*128 + p]
    hi, j = kt // 4, kt % 4
    return tT[:, j, 64 * hi : 64 * hi + 64]


def _build(ts=L):
    """Trace the per-core program (identical for all cores; SPMD).

    ts: first "late" step — from step ts on, every core has <= 32 active
    rows (lengths sorted desc per core), and the kernel switches to a
    stacked-4 layout [b + 32*(f//256), f%256] with 4-way column-tiled
    matmuls (4 concurrent 256-col weight streams -> 2x PE throughput).
    """
    nc = bass.Bass("TRN2", dynamic_dma_scratch_size=64)
    di = {}
    inputs = [
        ("vT", [128, 16, S], F16),
        ("qT", [128, 8, S], F16),
        ("xT2", [NPAIR, 128, 8, 128], F16),
        ("wvT", [16, 128, HD], F16),
        ("wqT", [8, 128, HD], F16),
        ("wihT", [128, 8, 3 * HD], F16),
        ("whhT", [128, 8, 3 * HD], F16),
        ("whT", [128, 8, HD], F16),
        ("wlT", [128, 8, HD], F16),
        ("wcihT", [128, 8, 3 * HD], F16),
        ("wcwhT", [128, 8, 3 * HD], F16),
        ("wfT", [128, 8, HD], F16),
    ]
    for name, shape, dt in inputs:
        di[name] = nc.dram_tensor(name, shape, dt, kind="ExternalInput")
    outs_d = nc.dram_tensor("outs", [L, 128, 512], F32, kind="ExternalOutput")
    alph_d = nc.dram_tensor("alph", [L, 128, 512], F16, kind="ExternalOutput")

    with tile.TileContext(nc) as tc:
        _trace(nc, tc, di, outs_d, alph_d, ts)
    _split_waits(nc)
    return nc


def _trace(nc, tc, di, outs_d, alph_d, ts):
    import contextlib

    ctx = contextlib.ExitStack()
    with ctx:
        # ---- resident weights, phase 1 ----
        res1 = ctx.enter_context(tc.tile_pool(name="res1", bufs=1))
        whh_t = res1.tile([128, 8, 3 * HD], F16, tag="whh")
        wh_t = res1.tile([128, 8, HD], F16, tag="wh")
        wl_t = res1.tile([128, 8, HD], F16, tag="wl")



        # ---- DRAM scratch for precomputed gi ----
        dram = ctx.enter_context(tc.tile_pool(name="dram", bufs=1, space="DRAM"))
        gi_dram = dram.tile([L, 128, 3, 512], F16, tag="gi_dram")

        # ---- work pool ----
        work = ctx.enter_context(tc.tile_pool(name="work", bufs=1))
        ctr = [0]

        def wtile(shape, dt, tag, bufs):
            def mk(shp=None):
                ctr[0] += 1
                return work.tile(shp or shape, dt, tag=tag, bufs=bufs,
                                 name=f"{tag}_{ctr[0]}",
                                 padded_shape=shape)
            return mk

        t_xt2 = wtile([128, 8, 128], F16, "xt2", 2)
        t_gi = wtile([128, 3, 512], F16, "gi", 2)
        t_tmp = wtile([128, 512], F16, "tmp", 4)
        t_act = wtile([128, 512], F16, "act", 5)
        t_rw = t_zw = t_nw = t_rc = t_zc = t_nct = t_act
        t_jrl = t_grc = t_att = t_act
        t_h1 = wtile([128, 512], F16, "h1", 1)
        t_h2 = wtile([128, 512], F16, "h2", 1)
        t_h1T = wtile([128, 4, 128], F16, "h1T", 1)
        t_h2T = wtile([128, 4, 128], F16, "h2T", 1)
        t_jT = wtile([128, 4, 128], F16, "jT", 1)
        t_attT = wtile([128, 4, 128], F16, "attT", 1)
        t_grcT = wtile([128, 4, 128], F16, "grcT", 1)
        t_cinT = wtile([128, 4, 128], F16, "cinT", 1)
        t_h2n = wtile([128, 512], F32, "h2n", 1)

        pvq_t = work.tile([128, 512], F16, tag="pvq")
        ident = work.tile([128, 128], F16, tag="ident")
        from concourse.masks import make_identity
        make_identity(nc, ident)

        # ---- prologue: pvq + batched gi -> DRAM ----
        with tc.tile_pool(name="pre", bufs=1) as pre:
            # sync ring: wih in per-kt chunks right behind xp0 — the first
            # gi pair needs kt 0 ~6us in. scalar ring: v/q/wvq (pvq path),
            # then res1 weights (needed from slot 0 on).
            wih_t = pre.tile([128, 8, 3 * HD], F16, tag="wih")
            v_t = pre.tile([128, 16, S], F16, tag="v")
            q_t = pre.tile([128, 8, S], F16, tag="q")
            nc.scalar.dma_start(out=v_t, in_=di["vT"][:])
            nc.scalar.dma_start(out=q_t, in_=di["qT"][:])

            def t_stage():
                ctr[0] += 1
                return pre.tile([128, 512], F16, tag="stage", bufs=4,
                                name=f"stage_{ctr[0]}")

            with tc.tile_pool(name="preps", bufs=1, space="PSUM") as preps:
                pv = preps.tile([128, 512], F32, tag="pv")
                for kt in range(16):
                    wc = pre.tile([128, HD], F16, tag="wvq", bufs=2,
                                  name=f"wv_{kt}")
                    nc.scalar.dma_start(out=wc, in_=di["wvT"][kt])
                    nc.tensor.matmul(pv[0:64, :], v_t[:, kt, :], wc[:, 0:512],
                                     start=(kt == 0), stop=False)
                    nc.tensor.matmul(pv[64:128, :], v_t[:, kt, :],
                                     wc[:, 512:1024], start=(kt == 0),
                                     stop=False)
                for kt in range(8):
                    wc = pre.tile([128, HD], F16, tag="wvq", bufs=2,
                                  name=f"wq_{kt}")
                    nc.scalar.dma_start(out=wc, in_=di["wqT"][kt])
                    nc.tensor.matmul(pv[0:64, :], q_t[:, kt, :], wc[:, 0:512],
                                     start=False, stop=(kt == 7))
                    nc.tensor.matmul(pv[64:128, :], q_t[:, kt, :],
                                     wc[:, 512:1024], start=False,
                                     stop=(kt == 7))
                nc.vector.tensor_copy(out=pvq_t, in_=pv)

                # batched gi: per pair of steps, 128-wide stationary
                xp0 = t_xt2()
                nc.sync.dma_start(out=xp0, in_=di["xT2"][0])
                for kt in range(8):
                    nc.sync.dma_start(out=wih_t[:, kt, :],
                                      in_=di["wihT"][:, kt, :])
                nc.scalar.dma_start(out=whh_t, in_=di["whhT"][:])
                nc.scalar.dma_start(out=wh_t, in_=di["whT"][:])
                nc.scalar.dma_start(out=wl_t, in_=di["wlT"][:])
                xp_prev = xp0
                for p in range(NPAIR):
                    xp = xp_prev
                    if p + 1 < NPAIR:
                        xp_prev = t_xt2()
                        nc.sync.dma_start(out=xp_prev, in_=di["xT2"][p + 1])
                    ps6 = []
                    for i in range(6):
                        ctr[0] += 1
                        ps6.append(preps.tile([128, 512], F32, tag="pg",
                                              bufs=7, name=f"pg_{ctr[0]}"))
                    for kt in range(8):
                        lhsT = xp[:, kt, :]
                        st, sp = kt == 0, kt == 7
                        for i in range(6):
                            nc.tensor.matmul(
                                ps6[i], lhsT,
                                wih_t[:, kt, 512 * i : 512 * (i + 1)],
                                start=st, stop=sp)
                    for g in range(3):
                        lo, hi = ps6[2 * g], ps6[2 * g + 1]
                        slo = t_stage()
                        nc.vector.tensor_copy(out=slo, in_=lo)
                        shi = t_stage()
                        nc.vector.tensor_copy(out=shi, in_=hi)
                        for s2 in range(2):
                            t = 2 * p + s2
                            nc.sync.dma_start(
                                out=gi_dram[t, 0:64, g, :],
                                in_=slo[64 * s2 : 64 * s2 + 64, :])
                            nc.sync.dma_start(
                                out=gi_dram[t, 64:128, g, :],
                                in_=shi[64 * s2 : 64 * s2 + 64, :])

        # ---- resident weights, phase 2 (after prologue pool freed) ----
        res2 = ctx.enter_context(tc.tile_pool(name="res2", bufs=1))
        wcwh_t = res2.tile([128, 8, 3 * HD], F16, tag="wcwh")
        wcih_t = res2.tile([128, 8, 3 * HD], F16, tag="wcih")
        wf_t = res2.tile([128, 8, HD], F16, tag="wf")

        # ---- main-loop PSUM pool ----
        psum = ctx.enter_context(tc.tile_pool(name="psum", bufs=1,
                                              space="PSUM"))

        def gate_ps(w=512):
            ctr[0] += 1
            return psum.tile([128, w], F32, tag="pg", bufs=5,
                             name=f"ps_{ctr[0]}", padded_shape=[128, 512])

        def rot_ps(w=512):
            ctr[0] += 1
            return psum.tile([128, w], F32, tag="rot", bufs=2,
                             name=f"rot_{ctr[0]}", padded_shape=[128, 512])

        def pt_ps():
            ctr[0] += 1
            return psum.tile([128, 512], F16, tag="psT", bufs=1,
                             name=f"psT_{ctr[0]}")

        def pe_transpose(dstT, src_f16, evac, late=False):
            # dstT [128, 4, 128] <- transpose of stacked src fp16.
            # early: src [128, 512] stacked-2, 4 full 128x128 transposes.
            # late: src [128, 256] stacked-4, 8 narrow [32,128] transposes
            # (only batch columns 64h..64h+32 of dstT become valid).
            pt = pt_ps()
            if not late:
                for j in range(4):
                    nc.tensor.transpose(
                        pt[:, 128 * j : 128 * (j + 1)],
                        src_f16[:, 128 * j : 128 * (j + 1)],
                        ident,
                    )
            else:
                for q in range(4):
                    for jj in range(2):
                        jt = 2 * (q % 2) + jj
                        off = 128 * jt + 64 * (q // 2)
                        nc.tensor.transpose(
                            pt[:, off : off + 32],
                            src_f16[32 * q : 32 * q + 32,
                                    128 * jj : 128 * jj + 128],
                            ident[32 * q : 32 * q + 32,
                                  32 * q : 32 * q + 32],
                        )
            if evac == "vector":
                nc.vector.tensor_copy(
                    out=dstT.rearrange("p j q -> p (j q)"), in_=pt)
            else:
                nc.scalar.activation(
                    out=dstT.rearrange("p j q -> p (j q)"), in_=pt, func=Copy)

        def pe_to_s4(dst_s4, srcT, evac):
            # dst_s4 [128, 256] <- stacked-4 view of feature-major srcT
            # [128, 4, 128] (batch 0..31 only); used once at the ts
            # boundary to convert carried state.
            pt = pt_ps()
            for q in range(4):
                for jj in range(2):
                    jt = 2 * (q % 2) + jj
                    h = q // 2
                    nc.tensor.transpose(
                        pt[32 * q : 32 * q + 32, 128 * jj : 128 * jj + 128],
                        srcT[:, jt, 64 * h : 64 * h + 32],
                        ident,
                    )
            if evac == "vector":
                nc.vector.tensor_copy(out=dst_s4, in_=pt[:, 0:256])
            else:
                nc.scalar.activation(out=dst_s4, in_=pt[:, 0:256], func=Copy)

        def mm_gates(psums, statT, w_t, start, stops, late=False):
            """Per ktile: col-tiled matmuls for each 1024-wide gate i of
            psums (accumulating over ktiles). `stops` is a per-psum list
            of whether the group closes at kt 7. Early: 64-wide
            stationary, T0/T1 pair of 512-col streams. Late: 32-wide
            stationary, 4 concurrent 256-col streams."""
            if stops in (True, False):
                stops = [stops] * len(psums)
            for kt in range(8):
                hi, j = kt // 4, kt % 4
                st = start and kt == 0
                if not late:
                    lhsT = statT[:, j, 64 * hi : 64 * hi + 64]
                    for i, ps in enumerate(psums):
                        sp = stops[i] and kt == 7
                        nc.tensor.matmul(
                            ps[0:64, :], lhsT,
                            w_t[:, kt, 1024 * i : 1024 * i + 512],
                            start=st, stop=sp)
                        nc.tensor.matmul(
                            ps[64:128, :], lhsT,
                            w_t[:, kt, 1024 * i + 512 : 1024 * i + 1024],
                            start=st, stop=sp)
                else:
                    lhsT = statT[:, j, 64 * hi : 64 * hi + 32]
                    for i, ps in enumerate(psums):
                        sp = stops[i] and kt == 7
                        for q in range(4):
                            c0 = 1024 * i + 256 * q
                            nc.tensor.matmul(
                                ps[32 * q : 32 * q + 32, :], lhsT,
                                w_t[:, kt, c0 : c0 + 256],
                                start=st, stop=sp)

        # ---- pvq in stacked-4 for late steps (DRAM bounce remap) ----
        pvq4 = None
        if ts < L:
            pvq_dram = dram.tile([128, 512], F16, tag="pvq_dram")
            nc.sync.dma_start(out=pvq_dram, in_=pvq_t)
            pvq4 = work.tile([128, 256], F16, tag="pvq4")
            nc.sync.dma_start(
                out=pvq4[0:64, :],
                in_=pvq_dram[0:32, :].rearrange("b (q c) -> q b c", q=2))
            nc.sync.dma_start(
                out=pvq4[64:128, :],
                in_=pvq_dram[64:96, :].rearrange("b (q c) -> q b c", q=2))

        # ---- initial state ----
        sw0 = [128, 256] if ts == 0 else [128, 512]
        h1 = t_h1(sw0)
        nc.vector.memset(h1, 0.0)
        h1T_prev = t_h1T()
        nc.vector.memset(h1T_prev, 0.0)
        h2 = t_h2(sw0)
        nc.vector.memset(h2, 0.0)
        h2T_prev = t_h2T()
        nc.vector.memset(h2T_prev, 0.0)

        gi_tiles, xp_tiles = {}, {}

        def load_gi(t):
            if t < ts:
                g = t_gi()
                nc.sync.dma_start(out=g, in_=gi_dram[t])
            else:
                g = t_gi([128, 3, 256])
                for gg in range(3):
                    nc.sync.dma_start(
                        out=g[0:64, gg, :],
                        in_=gi_dram[t, 0:32, gg, :].rearrange(
                            "b (q c) -> q b c", q=2))
                    nc.sync.dma_start(
                        out=g[64:128, gg, :],
                        in_=gi_dram[t, 64:96, gg, :].rearrange(
                            "b (q c) -> q b c", q=2))
            gi_tiles[t] = g

        def load_xp(p):
            xp = t_xt2()
            nc.sync.dma_start(out=xp, in_=di["xT2"][p])
            xp_tiles[p] = xp

        # gi(0)/xp(0) queued ahead of the phase-2 weight loads so the
        # first slot's inputs land first. Phase-2 weights stream in
        # per-ktile chunks across two rings so consumers (E/D/F of the
        # first c-slot) start as soon as their chunk lands.
        # phase-2 weights: whole-tensor DMAs on the scalar ring (behind
        # res1 there); sync ring stays clear for per-slot gi/xp/alph/outs
        load_gi(0)
        load_xp(0)
        nc.scalar.dma_start(out=wcwh_t, in_=di["wcwhT"][:])
        nc.scalar.dma_start(out=wcih_t, in_=di["wcihT"][:])
        nc.scalar.dma_start(out=wf_t, in_=di["wfT"][:])

        cin_prev = None  # cinT for step k-1
        c_ps = None      # (Rc, Zc, HNc) psums for c-step in flight

        # ---- main loop: slot k handles h1-chain step k, c-chain k-1 ----
        for k in range(L + 1):
            hk, ck = k, k - 1
            do_h, do_c = hk < L, ck >= 0
            lh, lc = hk >= ts, ck >= ts       # late (stacked-4) modes
            wh_ = 256 if lh else 512
            wc_ = 256 if lc else 512

            # prefetches for next slot
            if hk + 1 < L:
                load_gi(hk + 1)
                if (hk + 1) % 2 == 0:
                    load_xp((hk + 1) // 2)

            # === A(k): w-GRU hidden projections ===
            if do_h:
                Rw, Zw, HNw = gate_ps(wh_), gate_ps(wh_), gate_ps(wh_)
                mm_gates([Rw, Zw, HNw], h1T_prev, whh_t, True, True,
                         late=lh)

            # === E(ck): c-GRU hidden projections (group left open) ===
            if do_c:
                Rc, Zc, HNc = gate_ps(wc_), gate_ps(wc_), gate_ps(wc_)
                mm_gates([Rc, Zc, HNc], h2T_prev, wcwh_t, True,
                         [False, False, True], late=lc)

            # === V1(k): w-GRU combine ===
            if do_h:
                shp = [128, wh_]
                git = gi_tiles.pop(hk)
                rs = t_tmp(shp)
                nc.vector.tensor_add(out=rs, in0=Rw, in1=git[:, 0, :])
                zs = t_tmp(shp)
                nc.vector.tensor_add(out=zs, in0=Zw, in1=git[:, 1, :])
                rw = t_rw(shp)
                nc.scalar.activation(out=rw, in_=rs, func=Sig)
                zw = t_zw(shp)
                nc.scalar.activation(out=zw, in_=zs, func=Sig)
                t1 = t_tmp(shp)
                nc.vector.tensor_mul(out=t1, in0=rw, in1=HNw)
                t2 = t_tmp(shp)
                nc.vector.tensor_add(out=t2, in0=t1, in1=git[:, 2, :])
                nw = t_nw(shp)
                nc.scalar.activation(out=nw, in_=t2, func=Tanh)
                d = t_tmp(shp)
                nc.vector.tensor_sub(out=d, in0=h1, in1=nw)
                e = t_tmp(shp)
                nc.vector.tensor_mul(out=e, in0=zw, in1=d)
                h1n = t_h1(shp)
                nc.vector.tensor_add(out=h1n, in0=nw, in1=e)
                h1 = h1n

                # === T1(k) + B(k): joint = relu(pvq + h1 @ Wh.T) ===
                h1T_new = t_h1T()
                pe_transpose(h1T_new, h1, evac="vector", late=lh)
                joint = rot_ps(wh_)
                mm_gates([joint], h1T_new, wh_t, True, True, late=lh)

            # === D(ck): c-GRU input projections (close Rc/Zc groups) ===
            if do_c:
                INc = gate_ps(wc_)
                for kt in range(8):
                    hi, j = kt // 4, kt % 4
                    sp = kt == 7
                    if not lc:
                        lhsT = cin_prev[:, j, 64 * hi : 64 * hi + 64]
                        for i, (ps, st) in enumerate(
                                [(Rc, False), (Zc, False), (INc, kt == 0)]):
                            nc.tensor.matmul(
                                ps[0:64, :], lhsT,
                                wcih_t[:, kt, 1024 * i : 1024 * i + 512],
                                start=st, stop=sp)
                            nc.tensor.matmul(
                                ps[64:128, :], lhsT,
                                wcih_t[:, kt,
                                       1024 * i + 512 : 1024 * i + 1024],
                                start=st, stop=sp)
                    else:
                        lhsT = cin_prev[:, j, 64 * hi : 64 * hi + 32]
                        for i, (ps, st) in enumerate(
                                [(Rc, False), (Zc, False), (INc, kt == 0)]):
                            for q in range(4):
                                c0 = 1024 * i + 256 * q
                                nc.tensor.matmul(
                                    ps[32 * q : 32 * q + 32, :], lhsT,
                                    wcih_t[:, kt, c0 : c0 + 256],
                                    start=st, stop=sp)

            if do_h:
                # === V2(k) + T2(k) + C(k): att = sig(relu(...) @ Wl.T) ===
                ja = t_tmp([128, wh_])
                nc.vector.tensor_add(out=ja, in0=joint,
                                     in1=pvq4 if lh else pvq_t)
                jrl = t_jrl([128, wh_])
                nc.scalar.activation(out=jrl, in_=ja, func=Relu)
                jT = t_jT()
                pe_transpose(jT, jrl, evac="vector", late=lh)
                attp = rot_ps(wh_)
                mm_gates([attp], jT, wl_t, True, True, late=lh)

            # === V5(ck): c-GRU combine ===
            if do_c:
                shp = [128, wc_]
                rc = t_rc(shp)
                nc.scalar.activation(out=rc, in_=Rc, func=Sig)
                zc = t_zc(shp)
                nc.scalar.activation(out=zc, in_=Zc, func=Sig)
                t1c = t_tmp(shp)
                nc.vector.tensor_mul(out=t1c, in0=rc, in1=HNc)
                t2c = t_tmp(shp)
                nc.vector.tensor_add(out=t2c, in0=t1c, in1=INc)
                nct = t_nct(shp)
                nc.scalar.activation(out=nct, in_=t2c, func=Tanh)
                dc = t_tmp(shp)
                nc.vector.tensor_sub(out=dc, in0=h2, in1=nct)
                ec = t_tmp(shp)
                nc.vector.tensor_mul(out=ec, in0=zc, in1=dc)
                grc = t_grc(shp)
                nc.vector.tensor_add(out=grc, in0=nct, in1=ec)

            if do_h:
                # === V3(k) + T3(k) + V4(k): att, cin ===
                att = t_att()
                nc.scalar.activation(out=att, in_=attp, func=Sig)
                nc.sync.dma_start(out=alph_d[hk], in_=att)
                attT = t_attT()
                pe_transpose(attT, att, evac="scalar")
                cinT = t_cinT()
                xp = xp_tiles[hk // 2]
                s2 = hk % 2
                xs = xp[:, :, 64 * s2 : 64 * s2 + 64]
                xt_r = xs.rearrange("p (hi j) s -> p j hi s", hi=2, j=4)
                nc.vector.tensor_mul(
                    out=cinT.rearrange("p j (hi s) -> p j hi s", hi=2),
                    in0=attT.rearrange("p j (hi s) -> p j hi s", hi=2),
                    in1=xt_r,
                )
                cin_new = cinT

            # === T4(ck) + F(ck): h2n = grc @ Wf.T ===
            if do_c:
                grcT = t_grcT()
                pe_transpose(grcT, grc, evac="vector")
                h2np = rot_ps()
                mm_gates([h2np], grcT, wf_t, True, True)
                h2n_sb = t_h2n()
                nc.vector.tensor_copy(out=h2n_sb, in_=h2np)
                nc.sync.dma_start(out=outs_d[ck], in_=h2n_sb)
                if ck < L - 1:
                    h2n = t_h2()
                    nc.scalar.activation(out=h2n, in_=h2np, func=Copy)
                    h2 = h2n
                    h2Tn = t_h2T()
                    pe_transpose(h2Tn, h2, evac="scalar")
                    h2T_prev = h2Tn

            if do_h:
                h1T_prev = h1T_new
                cin_prev = cin_new


_CACHED = {}


def _get_nc():
    if "nc" not in _CACHED:
        _CACHED["nc"] = _build()
    return _CACHED["nc"]


def _wn(V, g):
    return V * (g / np.linalg.norm(V.astype(np.float64)).astype(np.float32))


def _plainT(W):
    # [out, in] -> [in//128, 128, out] fp16
    inf = W.shape[1]
    return np.ascontiguousarray(W.T.reshape(inf // 128, 128, W.shape[0])).astype(
        np.float16
    )


def _plain128(W):
    # [out, in] -> [128, in//128, out] fp16 (partition-major, 1 DMA)
    return np.ascontiguousarray(np.transpose(_plainT(W), (1, 0, 2)))


def _prep_in_maps(inp):
    cap_len = inp["cap_len"].astype(np.int32)
    order = np.argsort(-cap_len, kind="stable")

    for bname in ["av_b", "aq_b", "ah_b", "al_b", "fc_b",
                  "w_bih", "w_bhh", "c_bih", "c_bhh"]:
        assert not np.any(inp[bname]), f"nonzero bias {bname} unsupported"

    Wv = _wn(inp["av_V"], inp["av_g"])
    Wq = _wn(inp["aq_V"], inp["aq_g"])
    Wh = _wn(inp["ah_V"], inp["ah_g"])
    Wl = _wn(inp["al_V"], inp["al_g"])
    Wf = _wn(inp["fc_V"], inp["fc_g"])

    shared = dict(
        wvT=_plainT(Wv), wqT=_plainT(Wq),
        wihT=_plain128(inp["w_Wih"]),
        whhT=_plain128(inp["w_Whh"]),
        whT=_plain128(Wh), wlT=_plain128(Wl),
        wcihT=_plain128(inp["c_Wih"]), wcwhT=_plain128(inp["c_Whh"]),
        wfT=_plain128(Wf),
    )

    v, q, caption = inp["v"], inp["q"], inp["caption"]
    in_maps = []
    for k in range(NCORES):
        pos = np.arange(S) * NCORES + k  # sorted positions of this core
        vk = v[pos].astype(np.float16)            # [S, VD]
        qk = q[pos].astype(np.float16)
        capk = caption[order[pos]].astype(np.float16)  # [S, L, QD]
        m = dict(shared)
        m["vT"] = np.ascontiguousarray(
            np.transpose(vk.T.reshape(16, 128, S), (1, 0, 2)))
        m["qT"] = np.ascontiguousarray(
            np.transpose(qk.T.reshape(8, 128, S), (1, 0, 2)))
        # xT2[p, pf, kt, 64*s + b] = cap[b, 2p+s, 128*kt + pf]
        c2 = capk.reshape(S, NPAIR, 2, 8, 128)
        m["xT2"] = np.ascontiguousarray(
            np.transpose(c2, (1, 4, 3, 2, 0)).reshape(NPAIR, 128, 8, 128))
        in_maps.append(m)
    return in_maps


def kernel(**inputs):
    inp = {k: np.asarray(v) for k, v in inputs.items()}
    cap_len = inp["cap_len"].astype(np.int32)
    order = np.argsort(-cap_len, kind="stable")
    cl = cap_len[order]
    in_maps = _prep_in_maps(inp)

    nc = _get_nc()
    res = run_bass_kernel_spmd(nc, in_maps, core_ids=list(range(NCORES)))

    outs = np.zeros((B, L, HD), np.float32)
    alphas = np.zeros((B, L, HD), np.float32)
    for k in range(NCORES):
        pos = np.arange(S) * NCORES + k
        od = res.results[k]["outs"]  # [L, 128, 512] f32
        ad = res.results[k]["alph"].astype(np.float32)
        oc = np.concatenate([od[:, :S, :], od[:, S:, :]], axis=2)  # [L, S, HD]
        ac = np.concatenate([ad[:, :S, :], ad[:, S:, :]], axis=2)
        outs[pos] = np.transpose(oc, (1, 0, 2))
        alphas[pos] = np.transpose(ac, (1, 0, 2))

    mask = (np.arange(L)[None, :] < cl[:, None])[:, :, None]
    outs *= mask
    alphas *= mask
    return outs, alphas


# revision 46
# speedup vs baseline: 1.9418x; 1.1510x over previous
"""Trainium2 Bass kernel for nn_CaptionEmbedding (ragged double-GRU with
attention gating).

Strategy: data-parallel over batch across 8 cores (strided over the
length-sorted order so every core gets a balanced length mix). Per core a
fully-unrolled 20-step recurrence in fp16 (fp32 PSUM accumulation):
  - activations "stacked": [128, 512] = (slot + 64*feat_half, feat%512)
  - matmul stationary operands are activations (64-wide -> column-tiled
    T0/T1 pairs run concurrently); weights stream through the PE array
  - ALL weights resident in SBUF fp16 (whh, wcih, wcwh, wh, wl, wf)
  - w-GRU input projections (gi = x @ Wih^T) for all 20 steps are
    precomputed in the prologue with a 128-wide stationary (full PE
    utilization), bounced through a DRAM scratch, and streamed back one
    step per slot (3 KiB/partition)
  - the c-GRU chain (cgh/cgi/Wf) runs one step behind the w-GRU/attention
    chain so the PE has independent work during the recurrent combines
"""
import numpy as np

import concourse.bass as bass
import concourse.mybir as mybir
import concourse.tile as tile
from concourse.bass_utils import run_bass_kernel_spmd
import concourse.mybir as _mybir

B, VD, QD, HD, L = 512, 2048, 1024, 1024, 20
NCORES, S = 8, 64
NPAIR = L // 2
F32, F16 = mybir.dt.float32, mybir.dt.float16
Sig = mybir.ActivationFunctionType.Sigmoid
Tanh = mybir.ActivationFunctionType.Tanh
Relu = mybir.ActivationFunctionType.Relu
Copy = mybir.ActivationFunctionType.Copy

_MAX_WAITS = 1
_wait_ctr = [0]


def _dedupe_ldw(nc):
    """Tile legalization emits one InstLdweights per matmul; consecutive
    matmuls over the same stationary reload identical weights. Drop exact
    duplicates (no sync side effects) so the PE streams back-to-back."""
    import concourse.mybir as mb

    dropped = 0
    for fn in nc.m.functions:
        for bb in fn.blocks:
            out = []
            last = {}  # (tile_position, tile_size) -> weights key
            for inst in bb.instructions:
                nm = type(inst).__name__
                if nm == "InstLdweights":
                    si = inst.sync_info
                    pos = (
                        tuple(getattr(inst, "tile_position", None) or (-1,)),
                        tuple(getattr(inst, "tile_size", None) or (-1,)),
                    )
                    key = (
                        str(inst.ins[0]),
                        bool(getattr(inst, "is_transpose", False)),
                        str(getattr(inst, "perf_mode", None)),
                    )
                    clean = not (si and (si.on_wait or si.on_update))
                    if clean and last.get(pos) == key:
                        dropped += 1
                        continue
                    last[pos] = key
                elif nm == "InstMatmult":
                    pass
                elif inst.engine == mb.EngineType.PE:
                    last.clear()
                out.append(inst)
            if len(out) != len(bb.instructions):
                bb.instructions[:] = out
    return dropped


def _split_waits(nc):
    # container neuronxcc rejects >= 2 sync waits on one instruction; move
    # extras onto same-engine nops spliced just before it
    for fn in nc.m.functions:
        for bb in fn.blocks:
            out = []
            for inst in bb.instructions:
                si = inst.sync_info
                waits = list(si.on_wait) if si and si.on_wait else []
                if len(waits) > _MAX_WAITS:
                    extra, keep = waits[:-_MAX_WAITS], waits[-_MAX_WAITS:]
                    for i in range(0, len(extra), _MAX_WAITS):
                        _wait_ctr[0] += 1
                        nop = _mybir.InstNoOp(
                            name=f"waitsplit_nop_{_wait_ctr[0]}", ins=[], outs=[]
                        )
                        nop.engine = inst.engine
                        nop.sync_info = _mybir.SyncInfo(
                            on_wait=extra[i : i + _MAX_WAITS], on_update=[]
                        )
                        nc.register_instruction(nop)
                        out.append(nop)
                    si.on_wait = keep
                out.append(inst)
            if len(out) != len(bb.instructions):
                bb.instructions[:] = out


def _kt_slice(tT, kt):
    # stationary [128, 64] for feature ktile kt from a transposed
    # [128, 4, 128] tile: tT[p, j, q] = stacked[q, j*128 + p]
    hi, j = kt // 4, kt % 4
    return tT[:, j, 64 * hi : 64 * hi + 64]


def _build(ts=L):
    """Trace the per-core program (identical for all cores; SPMD).

    ts: first "late" step — from step ts on, every core has <= 32 active
    rows (lengths sorted desc per core), and the kernel switches to a
    stacked-4 layout [b + 32*(f//256), f%256] with 4-way column-tiled
    matmuls (4 concurrent 256-col weight streams -> 2x PE throughput).
    """
    nc = bass.Bass("TRN2", dynamic_dma_scratch_size=64)
    di = {}
    inputs = [
        ("vT", [128, 16, S], F16),
        ("qT", [128, 8, S], F16),
        ("xT2", [NPAIR, 128, 8, 128], F16),
        ("wvT", [16, 128, HD], F16),
        ("wqT", [8, 128, HD], F16),
        ("wihT", [128, 8, 3 * HD], F16),
        ("whhT", [128, 8, 3 * HD], F16),
        ("whT", [128, 8, HD], F16),
        ("wlT", [128, 8, HD], F16),
        ("wcihT", [128, 8, 3 * HD], F16),
        ("wcwhT", [128, 8, 3 * HD], F16),
        ("wfT", [128, 8, HD], F16),
    ]
    for name, shape, dt in inputs:
        di[name] = nc.dram_tensor(name, shape, dt, kind="ExternalInput")
    outs_d = nc.dram_tensor("outs", [L, 128, 512], F32, kind="ExternalOutput")
    alph_d = nc.dram_tensor("alph", [L, 128, 512], F16, kind="ExternalOutput")

    with tile.TileContext(nc) as tc:
        _trace(nc, tc, di, outs_d, alph_d, ts)
    _split_waits(nc)
    return nc


def _trace(nc, tc, di, outs_d, alph_d, ts):
    import contextlib

    ctx = contextlib.ExitStack()
    with ctx:
        # ---- resident weights, phase 1 ----
        res1 = ctx.enter_context(tc.tile_pool(name="res1", bufs=1))
        whh_t = res1.tile([128, 8, 3 * HD], F16, tag="whh")
        wh_t = res1.tile([128, 8, HD], F16, tag="wh")
        wl_t = res1.tile([128, 8, HD], F16, tag="wl")



        # ---- DRAM scratch for precomputed gi ----
        dram = ctx.enter_context(tc.tile_pool(name="dram", bufs=1, space="DRAM"))
        gi_dram = dram.tile([L, 128, 3, 512], F16, tag="gi_dram")

        # ---- work pool ----
        work = ctx.enter_context(tc.tile_pool(name="work", bufs=1))
        ctr = [0]

        def wtile(shape, dt, tag, bufs):
            def mk(shp=None):
                ctr[0] += 1
                return work.tile(shp or shape, dt, tag=tag, bufs=bufs,
                                 name=f"{tag}_{ctr[0]}",
                                 padded_shape=shape)
            return mk

        t_xt2 = wtile([128, 8, 128], F16, "xt2", 2)
        t_gi = wtile([128, 3, 512], F16, "gi", 2)
        t_tmp = wtile([128, 512], F16, "tmp", 4)
        t_act = wtile([128, 512], F16, "act", 5)
        t_rw = t_zw = t_nw = t_rc = t_zc = t_nct = t_act
        t_jrl = t_grc = t_att = t_act
        t_h1 = wtile([128, 512], F16, "h1", 1)
        t_h2 = wtile([128, 512], F16, "h2", 1)
        t_h1T = wtile([128, 4, 128], F16, "h1T", 1)
        t_h2T = wtile([128, 4, 128], F16, "h2T", 1)
        t_jT = wtile([128, 4, 128], F16, "jT", 1)
        t_attT = wtile([128, 4, 128], F16, "attT", 1)
        t_grcT = wtile([128, 4, 128], F16, "grcT", 1)
        t_cinT = wtile([128, 4, 128], F16, "cinT", 1)
        t_h2n = wtile([128, 512], F32, "h2n", 1)

        pvq_t = work.tile([128, 512], F16, tag="pvq")
        ident = work.tile([128, 128], F16, tag="ident")
        from concourse.masks import make_identity
        make_identity(nc, ident)

        # ---- prologue: pvq + batched gi -> DRAM ----
        with tc.tile_pool(name="pre", bufs=1) as pre:
            # sync ring: wih in per-kt chunks right behind xp0 — the first
            # gi pair needs kt 0 ~6us in. scalar ring: v/q/wvq (pvq path),
            # then res1 weights (needed from slot 0 on).
            wih_t = pre.tile([128, 8, 3 * HD], F16, tag="wih")
            v_t = pre.tile([128, 16, S], F16, tag="v")
            q_t = pre.tile([128, 8, S], F16, tag="q")
            nc.scalar.dma_start(out=v_t, in_=di["vT"][:])
            nc.scalar.dma_start(out=q_t, in_=di["qT"][:])

            def t_stage():
                ctr[0] += 1
                return pre.tile([128, 512], F16, tag="stage", bufs=4,
                                name=f"stage_{ctr[0]}")

            with tc.tile_pool(name="preps", bufs=1, space="PSUM") as preps:
                pv = preps.tile([128, 512], F32, tag="pv")
                for kt in range(16):
                    wc = pre.tile([128, HD], F16, tag="wvq", bufs=2,
                                  name=f"wv_{kt}")
                    nc.scalar.dma_start(out=wc, in_=di["wvT"][kt])
                    nc.tensor.matmul(pv[0:64, :], v_t[:, kt, :], wc[:, 0:512],
                                     start=(kt == 0), stop=False)
                    nc.tensor.matmul(pv[64:128, :], v_t[:, kt, :],
                                     wc[:, 512:1024], start=(kt == 0),
                                     stop=False)
                for kt in range(8):
                    wc = pre.tile([128, HD], F16, tag="wvq", bufs=2,
                                  name=f"wq_{kt}")
                    nc.scalar.dma_start(out=wc, in_=di["wqT"][kt])
                    nc.tensor.matmul(pv[0:64, :], q_t[:, kt, :], wc[:, 0:512],
                                     start=False, stop=(kt == 7))
                    nc.tensor.matmul(pv[64:128, :], q_t[:, kt, :],
                                     wc[:, 512:1024], start=False,
                                     stop=(kt == 7))
                nc.vector.tensor_copy(out=pvq_t, in_=pv)

                # batched gi: per pair of steps, 128-wide stationary
                xp0 = t_xt2()
                nc.sync.dma_start(out=xp0, in_=di["xT2"][0])
                for kt in range(8):
                    nc.sync.dma_start(out=wih_t[:, kt, :],
                                      in_=di["wihT"][:, kt, :])
                nc.scalar.dma_start(out=whh_t, in_=di["whhT"][:])
                nc.scalar.dma_start(out=wh_t, in_=di["whT"][:])
                nc.scalar.dma_start(out=wl_t, in_=di["wlT"][:])
                xp_prev = xp0
                for p in range(NPAIR):
                    xp = xp_prev
                    if p + 1 < NPAIR:
                        xp_prev = t_xt2()
                        nc.sync.dma_start(out=xp_prev, in_=di["xT2"][p + 1])
                    ps6 = []
                    for i in range(6):
                        ctr[0] += 1
                        ps6.append(preps.tile([128, 512], F32, tag="pg",
                                              bufs=7, name=f"pg_{ctr[0]}"))
                    for kt in range(8):
                        lhsT = xp[:, kt, :]
                        st, sp = kt == 0, kt == 7
                        for i in range(6):
                            nc.tensor.matmul(
                                ps6[i], lhsT,
                                wih_t[:, kt, 512 * i : 512 * (i + 1)],
                                start=st, stop=sp)
                    for g in range(3):
                        lo, hi = ps6[2 * g], ps6[2 * g + 1]
                        slo = t_stage()
                        nc.vector.tensor_copy(out=slo, in_=lo)
                        shi = t_stage()
                        nc.vector.tensor_copy(out=shi, in_=hi)
                        for s2 in range(2):
                            t = 2 * p + s2
                            nc.sync.dma_start(
                                out=gi_dram[t, 0:64, g, :],
                                in_=slo[64 * s2 : 64 * s2 + 64, :])
                            nc.sync.dma_start(
                                out=gi_dram[t, 64:128, g, :],
                                in_=shi[64 * s2 : 64 * s2 + 64, :])

        # ---- resident weights, phase 2 (after prologue pool freed) ----
        res2 = ctx.enter_context(tc.tile_pool(name="res2", bufs=1))
        wcwh_t = res2.tile([128, 8, 3 * HD], F16, tag="wcwh")
        wcih_t = res2.tile([128, 8, 3 * HD], F16, tag="wcih")
        wf_t = res2.tile([128, 8, HD], F16, tag="wf")

        # ---- main-loop PSUM pool ----
        psum = ctx.enter_context(tc.tile_pool(name="psum", bufs=1,
                                              space="PSUM"))

        def gate_ps(w=512):
            ctr[0] += 1
            return psum.tile([128, w], F32, tag="pg", bufs=5,
                             name=f"ps_{ctr[0]}", padded_shape=[128, 512])

        def rot_ps(w=512):
            ctr[0] += 1
            return psum.tile([128, w], F32, tag="rot", bufs=2,
                             name=f"rot_{ctr[0]}", padded_shape=[128, 512])

        def pt_ps():
            ctr[0] += 1
            return psum.tile([128, 512], F16, tag="psT", bufs=1,
                             name=f"psT_{ctr[0]}")

        def pe_transpose(dstT, src_f16, evac, late=False):
            # dstT [128, 4, 128] <- transpose of stacked src fp16.
            # early: src [128, 512] stacked-2, 4 full 128x128 transposes.
            # late: src [128, 256] stacked-4, 8 narrow [32,128] transposes
            # (only batch columns 64h..64h+32 of dstT become valid).
            pt = pt_ps()
            nj = 4 if not late else 2
            for j in range(nj):
                nc.tensor.transpose(
                    pt[:, 128 * j : 128 * (j + 1)],
                    src_f16[:, 128 * j : 128 * (j + 1)],
                    ident,
                )
            if late:
                pt = pt[:, 0:256]
            if evac == "vector":
                nc.vector.tensor_copy(
                    out=dstT.rearrange("p j q -> p (j q)"), in_=pt)
            else:
                nc.scalar.activation(
                    out=dstT.rearrange("p j q -> p (j q)"), in_=pt, func=Copy)

        def mm_gates(psums, statT, w_t, start, stops, late=False,
                     fmt="qb"):
            """Per ktile: col-tiled matmuls for each 1024-wide gate i of
            psums (accumulating over ktiles). `stops` is a per-psum list
            of whether the group closes at kt 7. Early: 64-wide
            stationary, T0/T1 pair of 512-col streams. Late: 32-wide
            stationary, 4 concurrent 256-col streams."""
            if stops in (True, False):
                stops = [stops] * len(psums)
            for kt in range(8):
                hi, j = kt // 4, kt % 4
                st = start and kt == 0
                if not late:
                    lhsT = statT[:, j, 64 * hi : 64 * hi + 64]
                    for i, ps in enumerate(psums):
                        sp = stops[i] and kt == 7
                        nc.tensor.matmul(
                            ps[0:64, :], lhsT,
                            w_t[:, kt, 1024 * i : 1024 * i + 512],
                            start=st, stop=sp)
                        nc.tensor.matmul(
                            ps[64:128, :], lhsT,
                            w_t[:, kt, 1024 * i + 512 : 1024 * i + 1024],
                            start=st, stop=sp)
                else:
                    if fmt == "f4":
                        lhsT = statT[:, j, 64 * hi : 64 * hi + 32]
                    else:
                        lhsT = statT[:, kt % 2,
                                     32 * (kt // 2) : 32 * (kt // 2) + 32]
                    for i, ps in enumerate(psums):
                        sp = stops[i] and kt == 7
                        for q in range(4):
                            c0 = 1024 * i + 256 * q
                            nc.tensor.matmul(
                                ps[32 * q : 32 * q + 32, :], lhsT,
                                w_t[:, kt, c0 : c0 + 256],
                                start=st, stop=sp,
                                tile_position=(0, 32 * q))

        # ---- pvq in stacked-4 for late steps (DRAM bounce remap) ----
        pvq4 = None
        if ts < L:
            pvq_dram = dram.tile([128, 512], F16, tag="pvq_dram")
            nc.sync.dma_start(out=pvq_dram, in_=pvq_t)
            pvq4 = work.tile([128, 256], F16, tag="pvq4")
            nc.sync.dma_start(
                out=pvq4[0:64, :],
                in_=pvq_dram[0:32, :].rearrange("b (q c) -> q b c", q=2))
            nc.sync.dma_start(
                out=pvq4[64:128, :],
                in_=pvq_dram[64:96, :].rearrange("b (q c) -> q b c", q=2))

        # ---- initial state ----
        sw0 = [128, 256] if ts == 0 else [128, 512]
        h1 = t_h1(sw0)
        nc.vector.memset(h1, 0.0)
        h1T_prev = t_h1T()
        nc.vector.memset(h1T_prev, 0.0)
        h2 = t_h2(sw0)
        nc.vector.memset(h2, 0.0)
        h2T_prev = t_h2T()
        nc.vector.memset(h2T_prev, 0.0)

        gi_tiles, xp_tiles = {}, {}

        def load_gi(t):
            if t < ts:
                g = t_gi()
                nc.sync.dma_start(out=g, in_=gi_dram[t])
            else:
                g = t_gi([128, 3, 256])
                for gg in range(3):
                    nc.sync.dma_start(
                        out=g[0:64, gg, :],
                        in_=gi_dram[t, 0:32, gg, :].rearrange(
                            "b (q c) -> q b c", q=2))
                    nc.sync.dma_start(
                        out=g[64:128, gg, :],
                        in_=gi_dram[t, 64:96, gg, :].rearrange(
                            "b (q c) -> q b c", q=2))
            gi_tiles[t] = g

        def load_xp(p):
            xp = t_xt2()
            nc.sync.dma_start(out=xp, in_=di["xT2"][p])
            xp_tiles[p] = xp

        # gi(0)/xp(0) queued ahead of the phase-2 weight loads so the
        # first slot's inputs land first. Phase-2 weights stream in
        # per-ktile chunks across two rings so consumers (E/D/F of the
        # first c-slot) start as soon as their chunk lands.
        # phase-2 weights: whole-tensor DMAs on the scalar ring (behind
        # res1 there); sync ring stays clear for per-slot gi/xp/alph/outs
        load_gi(0)
        load_xp(0)
        nc.scalar.dma_start(out=wcwh_t, in_=di["wcwhT"][:])
        nc.scalar.dma_start(out=wcih_t, in_=di["wcihT"][:])
        nc.scalar.dma_start(out=wf_t, in_=di["wfT"][:])

        cin_prev = None  # cinT for step k-1
        c_ps = None      # (Rc, Zc, HNc) psums for c-step in flight

        # ---- main loop: slot k handles h1-chain step k, c-chain k-1 ----
        for k in range(L + 1):
            hk, ck = k, k - 1
            do_h, do_c = hk < L, ck >= 0
            lh, lc = hk >= ts, ck >= ts       # late (stacked-4) modes
            wh_ = 256 if lh else 512
            wc_ = 256 if lc else 512

            # prefetches for next slot
            if hk + 1 < L:
                load_gi(hk + 1)
                if (hk + 1) % 2 == 0:
                    load_xp((hk + 1) // 2)

            # === A(k): w-GRU hidden projections ===
            if do_h:
                Rw, Zw, HNw = gate_ps(wh_), gate_ps(wh_), gate_ps(wh_)
                mm_gates([Rw, Zw, HNw], h1T_prev, whh_t, True, True,
                         late=lh, fmt="f4" if hk == ts else "qb")

            # === E(ck): c-GRU hidden projections (group left open) ===
            if do_c:
                Rc, Zc, HNc = gate_ps(wc_), gate_ps(wc_), gate_ps(wc_)
                mm_gates([Rc, Zc, HNc], h2T_prev, wcwh_t, True,
                         [False, False, True], late=lc,
                         fmt="f4" if ck == ts else "qb")

            # === V1(k): w-GRU combine ===
            if do_h:
                shp = [128, wh_]
                git = gi_tiles.pop(hk)
                rs = t_tmp(shp)
                nc.vector.tensor_add(out=rs, in0=Rw, in1=git[:, 0, :])
                zs = t_tmp(shp)
                nc.vector.tensor_add(out=zs, in0=Zw, in1=git[:, 1, :])
                rw = t_rw(shp)
                nc.scalar.activation(out=rw, in_=rs, func=Sig)
                zw = t_zw(shp)
                nc.scalar.activation(out=zw, in_=zs, func=Sig)
                t1 = t_tmp(shp)
                nc.vector.tensor_mul(out=t1, in0=rw, in1=HNw)
                t2 = t_tmp(shp)
                nc.vector.tensor_add(out=t2, in0=t1, in1=git[:, 2, :])
                nw = t_nw(shp)
                nc.scalar.activation(out=nw, in_=t2, func=Tanh)
                d = t_tmp(shp)
                nc.vector.tensor_sub(out=d, in0=h1, in1=nw)
                e = t_tmp(shp)
                nc.vector.tensor_mul(out=e, in0=zw, in1=d)
                h1n = t_h1(shp)
                nc.vector.tensor_add(out=h1n, in0=nw, in1=e)
                h1 = h1n

                # === T1(k) + B(k): joint = relu(pvq + h1 @ Wh.T) ===
                h1T_new = t_h1T([128, 2, 128] if lh else None)
                pe_transpose(h1T_new, h1, evac="vector", late=lh)
                joint = rot_ps(wh_)
                mm_gates([joint], h1T_new, wh_t, True, True, late=lh)

            # === D(ck): c-GRU input projections (close Rc/Zc groups) ===
            if do_c:
                INc = gate_ps(wc_)
                for kt in range(8):
                    hi, j = kt // 4, kt % 4
                    sp = kt == 7
                    if not lc:
                        lhsT = cin_prev[:, j, 64 * hi : 64 * hi + 64]
                        for i, (ps, st) in enumerate(
                                [(Rc, False), (Zc, False), (INc, kt == 0)]):
                            nc.tensor.matmul(
                                ps[0:64, :], lhsT,
                                wcih_t[:, kt, 1024 * i : 1024 * i + 512],
                                start=st, stop=sp)
                            nc.tensor.matmul(
                                ps[64:128, :], lhsT,
                                wcih_t[:, kt,
                                       1024 * i + 512 : 1024 * i + 1024],
                                start=st, stop=sp)
                    else:
                        lhsT = cin_prev[:, kt % 2,
                                        32 * (kt // 2) : 32 * (kt // 2) + 32]
                        for i, (ps, st) in enumerate(
                                [(Rc, False), (Zc, False), (INc, kt == 0)]):
                            for q in range(4):
                                c0 = 1024 * i + 256 * q
                                nc.tensor.matmul(
                                    ps[32 * q : 32 * q + 32, :], lhsT,
                                    wcih_t[:, kt, c0 : c0 + 256],
                                    start=st, stop=sp,
                                    tile_position=(0, 32 * q))

            if do_h:
                # === V2(k) + T2(k) + C(k): att = sig(relu(...) @ Wl.T) ===
                ja = t_tmp([128, wh_])
                nc.vector.tensor_add(out=ja, in0=joint,
                                     in1=pvq4 if lh else pvq_t)
                jrl = t_jrl([128, wh_])
                nc.scalar.activation(out=jrl, in_=ja, func=Relu)
                jT = t_jT([128, 2, 128] if lh else None)
                pe_transpose(jT, jrl, evac="vector", late=lh)
                attp = rot_ps(wh_)
                mm_gates([attp], jT, wl_t, True, True, late=lh)

            # === V5(ck): c-GRU combine ===
            if do_c:
                shp = [128, wc_]
                rc = t_rc(shp)
                nc.scalar.activation(out=rc, in_=Rc, func=Sig)
                zc = t_zc(shp)
                nc.scalar.activation(out=zc, in_=Zc, func=Sig)
                t1c = t_tmp(shp)
                nc.vector.tensor_mul(out=t1c, in0=rc, in1=HNc)
                t2c = t_tmp(shp)
                nc.vector.tensor_add(out=t2c, in0=t1c, in1=INc)
                nct = t_nct(shp)
                nc.scalar.activation(out=nct, in_=t2c, func=Tanh)
                dc = t_tmp(shp)
                nc.vector.tensor_sub(out=dc, in0=h2, in1=nct)
                ec = t_tmp(shp)
                nc.vector.tensor_mul(out=ec, in0=zc, in1=dc)
                grc = t_grc(shp)
                nc.vector.tensor_add(out=grc, in0=nct, in1=ec)

            if do_h:
                # === V3(k) + T3(k) + V4(k): att, cin ===
                att = t_att([128, wh_])
                nc.scalar.activation(out=att, in_=attp, func=Sig)
                if not lh:
                    nc.sync.dma_start(out=alph_d[hk], in_=att)
                else:
                    nc.sync.dma_start(
                        out=alph_d[hk, 0:32, :].rearrange(
                            "b (q c) -> q b c", q=2),
                        in_=att[0:64, :])
                    nc.sync.dma_start(
                        out=alph_d[hk, 64:96, :].rearrange(
                            "b (q c) -> q b c", q=2),
                        in_=att[64:128, :])
                attT = t_attT([128, 2, 128] if lh else None)
                pe_transpose(attT, att, evac="scalar", late=lh)
                xp = xp_tiles[hk // 2]
                s2 = hk % 2
                if not lh:
                    cinT = t_cinT()
                    xs = xp[:, :, 64 * s2 : 64 * s2 + 64]
                    xt_r = xs.rearrange("p (hi j) s -> p j hi s", hi=2, j=4)
                    nc.vector.tensor_mul(
                        out=cinT.rearrange("p j (hi s) -> p j hi s", hi=2),
                        in0=attT.rearrange("p j (hi s) -> p j hi s", hi=2),
                        in1=xt_r,
                    )
                else:
                    # qb-major: cinT[p, cb, 32q + b] = attT * x(128*(2q+cb)+p, b)
                    cinT = t_cinT([128, 2, 128])
                    xs = xp[:, :, 64 * s2 : 64 * s2 + 32]
                    xt_r = xs.rearrange("p (q cb) b -> p cb q b", q=4, cb=2)
                    nc.vector.tensor_mul(
                        out=cinT.rearrange("p cb (q b) -> p cb q b", q=4),
                        in0=attT.rearrange("p cb (q b) -> p cb q b", q=4),
                        in1=xt_r,
                    )
                cin_new = cinT

            # === T4(ck) + F(ck): h2n = grc @ Wf.T ===
            if do_c:
                grcT = t_grcT([128, 2, 128] if lc else None)
                pe_transpose(grcT, grc, evac="vector", late=lc)
                h2np = rot_ps(wc_)
                mm_gates([h2np], grcT, wf_t, True, True, late=lc)
                h2n_sb = t_h2n([128, wc_])
                nc.vector.tensor_copy(out=h2n_sb, in_=h2np)
                if not lc:
                    nc.sync.dma_start(out=outs_d[ck], in_=h2n_sb)
                else:
                    nc.sync.dma_start(
                        out=outs_d[ck, 0:32, :].rearrange(
                            "b (q c) -> q b c", q=2),
                        in_=h2n_sb[0:64, :])
                    nc.sync.dma_start(
                        out=outs_d[ck, 64:96, :].rearrange(
                            "b (q c) -> q b c", q=2),
                        in_=h2n_sb[64:128, :])
                if ck < L - 1:
                    h2n = t_h2([128, wc_])
                    nc.scalar.activation(out=h2n, in_=h2np, func=Copy)
                    h2 = h2n
                    h2Tn = t_h2T([128, 2, 128] if lc else None)
                    pe_transpose(h2Tn, h2, evac="scalar", late=lc)
                    h2T_prev = h2Tn
                    if ck == ts - 1:
                        # boundary: carried c-state to stacked-4 via DRAM
                        hb_d = dram.tile([128, 512], F16, tag="h2b_d")
                        nc.sync.dma_start(out=hb_d, in_=h2)
                        h2b = t_h2([128, 256])
                        nc.sync.dma_start(
                            out=h2b[0:64, :],
                            in_=hb_d[0:32, :].rearrange(
                                "b (q c) -> q b c", q=2))
                        nc.sync.dma_start(
                            out=h2b[64:128, :],
                            in_=hb_d[64:96, :].rearrange(
                                "b (q c) -> q b c", q=2))
                        h2 = h2b

            if do_h:
                if hk == ts - 1:
                    # boundary: carried w-state to stacked-4 via DRAM
                    ha_d = dram.tile([128, 512], F16, tag="h1b_d")
                    nc.sync.dma_start(out=ha_d, in_=h1)
                    h1b = t_h1([128, 256])
                    nc.sync.dma_start(
                        out=h1b[0:64, :],
                        in_=ha_d[0:32, :].rearrange("b (q c) -> q b c", q=2))
                    nc.sync.dma_start(
                        out=h1b[64:128, :],
                        in_=ha_d[64:96, :].rearrange("b (q c) -> q b c", q=2))
                    h1 = h1b
                h1T_prev = h1T_new
                cin_prev = cin_new


_CACHED = {}


def _get_nc(ts=None):
    if ts is None:
        ts = _CACHED.get("last_ts", L)
    _CACHED["last_ts"] = ts
    key = ("nc", ts)
    if key not in _CACHED:
        _CACHED[key] = _build(ts)
    return _CACHED[key]


def _wn(V, g):
    return V * (g / np.linalg.norm(V.astype(np.float64)).astype(np.float32))


def _plainT(W):
    # [out, in] -> [in//128, 128, out] fp16
    inf = W.shape[1]
    return np.ascontiguousarray(W.T.reshape(inf // 128, 128, W.shape[0])).astype(
        np.float16
    )


def _plain128(W):
    # [out, in] -> [128, in//128, out] fp16 (partition-major, 1 DMA)
    return np.ascontiguousarray(np.transpose(_plainT(W), (1, 0, 2)))


def _prep_in_maps(inp):
    cap_len = inp["cap_len"].astype(np.int32)
    order = np.argsort(-cap_len, kind="stable")

    for bname in ["av_b", "aq_b", "ah_b", "al_b", "fc_b",
                  "w_bih", "w_bhh", "c_bih", "c_bhh"]:
        assert not np.any(inp[bname]), f"nonzero bias {bname} unsupported"

    Wv = _wn(inp["av_V"], inp["av_g"])
    Wq = _wn(inp["aq_V"], inp["aq_g"])
    Wh = _wn(inp["ah_V"], inp["ah_g"])
    Wl = _wn(inp["al_V"], inp["al_g"])
    Wf = _wn(inp["fc_V"], inp["fc_g"])

    shared = dict(
        wvT=_plainT(Wv), wqT=_plainT(Wq),
        wihT=_plain128(inp["w_Wih"]),
        whhT=_plain128(inp["w_Whh"]),
        whT=_plain128(Wh), wlT=_plain128(Wl),
        wcihT=_plain128(inp["c_Wih"]), wcwhT=_plain128(inp["c_Whh"]),
        wfT=_plain128(Wf),
    )

    v, q, caption = inp["v"], inp["q"], inp["caption"]
    in_maps = []
    for k in range(NCORES):
        pos = np.arange(S) * NCORES + k  # sorted positions of this core
        vk = v[pos].astype(np.float16)            # [S, VD]
        qk = q[pos].astype(np.float16)
        capk = caption[order[pos]].astype(np.float16)  # [S, L, QD]
        m = dict(shared)
        m["vT"] = np.ascontiguousarray(
            np.transpose(vk.T.reshape(16, 128, S), (1, 0, 2)))
        m["qT"] = np.ascontiguousarray(
            np.transpose(qk.T.reshape(8, 128, S), (1, 0, 2)))
        # xT2[p, pf, kt, 64*s + b] = cap[b, 2p+s, 128*kt + pf]
        c2 = capk.reshape(S, NPAIR, 2, 8, 128)
        m["xT2"] = np.ascontiguousarray(
            np.transpose(c2, (1, 4, 3, 2, 0)).reshape(NPAIR, 128, 8, 128))
        in_maps.append(m)
    return in_maps


def kernel(**inputs):
    inp = {k: np.asarray(v) for k, v in inputs.items()}
    cap_len = inp["cap_len"].astype(np.int32)
    order = np.argsort(-cap_len, kind="stable")
    cl = cap_len[order]
    in_maps = _prep_in_maps(inp)

    # first step at which every core's active-row count is <= 32
    # (per-core rows are the sorted order decimated by NCORES)
    ts = L
    for t in range(L):
        if all(int((cl[np.arange(S) * NCORES + k] > t).sum()) <= 32
               for k in range(NCORES)):
            ts = t
            break

    nc = _get_nc(ts)
    res = run_bass_kernel_spmd(nc, in_maps, core_ids=list(range(NCORES)))

    outs = np.zeros((B, L, HD), np.float32)
    alphas = np.zeros((B, L, HD), np.float32)
    for k in range(NCORES):
        pos = np.arange(S) * NCORES + k
        od = res.results[k]["outs"]  # [L, 128, 512] f32
        ad = res.results[k]["alph"].astype(np.float32)
        oc = np.concatenate([od[:, :S, :], od[:, S:, :]], axis=2)  # [L, S, HD]
        ac = np.concatenate([ad[:, :S, :], ad[:, S:, :]], axis=2)
        outs[pos] = np.transpose(oc, (1, 0, 2))
        alphas[pos] = np.transpose(ac, (1, 0, 2))

    mask = (np.arange(L)[None, :] < cl[:, None])[:, :, None]
    outs *= mask
    alphas *= mask
    return outs, alphas
